# revision 53
# baseline (speedup 1.0000x reference)
"""Causal self-attention (B=4, S=2048, D=1024, H=16) on 8 Trainium2 cores.

Sharding: core c -> (batch b = c//2, head-group g = c%2 of 8 heads).
Each core computes the QKV projection for its 8 heads, causal attention,
and a partial output projection (row-slice of w_proj); the host sums the
two partials per batch and adds biases.

Written in raw Bass (explicit engine programs + semaphores) because the
walrus build in this environment rejects instructions carrying more than
one sync-wait command, which Tile-generated BIR routinely produces.
Cross-engine dependencies use four counting semaphores, with waits
emitted as standalone wait_ge instructions:
  dq: DMA completions (+16 each)   pc: PE unit completions (+1)
  ca: ACT ops (+1)                 dv: DVE ops (+1)

Device-side dataflow (per core):
  - q^T/k^T [feat, token] come straight out of the projection (weights as
    lhsT, x^T as moving operand); heads are paired so a 2x64-dim head
    pair fills one 128-partition tile
  - v is computed in natural [token, feat] orientation (x^T tiles as
    lhsT, wv as moving operand) -- no transposes anywhere; the v-bias
    contribution folds into a host-side output correction
  - scores are computed transposed: s^T[key, query], head pair packed on
    the PE array via row tile_position (0,0)/(64,0); the softmax sum over
    keys is then a matmul contraction
  - ACT does exp(s*0.125) from PSUM into bf16 SBUF tiles; DVE applies the
    causal mask multiplicatively on diagonal blocks
  - the AV matmul uses ones-augmented v (lhsT [128 keys, 65]) so the
    softmax denominator lands in row 64 of the same PSUM accumulator
  - y is normalized via reciprocal + a K=1 replicate matmul, then the
    output projection accumulates over the 4 head-pair blocks
All matmuls take bf16 inputs with fp32 PSUM accumulation; softmax math
is fp32 except the reciprocal denominators, which are stored bf16 so
the bc replicate matmul streams at full PE rate (fp32 moving operands
run 4x slower).

Scheduling (vs. the original all-phases-serial version):
  - loads are one DMA per tensor group with a dedicated counting
    semaphore each (no SP self-gating), ordered so qk(pair 0) can start
    after ~3MB instead of the full 8MB
  - v(0..3) run up front through the psav double buffer; v(4..15)
    accumulate in ps6 one matmul at a time, sprinkled into attention
    kt-slots at a rate that lands each block just before its consumer
    (the last two blocks are held for tq=3, whose ACT-bound slots
    otherwise idle the PE)
  - bc replicates write rows 64:128 of the psav banks (free there:
    av uses 0:65 and recip has already read row 64), keeping ps7 free
    for projections
  - projections of tq-1 ride tq's kt loops 4 slots apart; the final 8
    alternate ps7/ps6 with a 3-deep osb store pipeline

PSUM banks: 0-3 ACT-consumed rotation (qk groups + score tiles),
4-5 AV accumulators + bc replicates (rows 64:128), 6 v-staging then
tail projections, 7 projections.
"""

import os
import sys

sys.path.insert(0, "/opt/trn_rl_repo")

import ml_dtypes
import numpy as np

B, S, D, H = 4, 2048, 1024, 16
HD = D // H          # 64
HPC = H // 2         # 8 heads per core
PD = 512             # local proj contraction (8 heads * 64)
P = 128
NTQ = S // 512       # 4 query tiles of 512
NKT = S // P         # 16 key tiles of 128
NPAIR = 4
DK = D // P          # 8 contraction tiles
N_ESLOT = 6

# DMA completion semaphores: one per load group so loads never serialize on
# SP self-gates (completions reordering across groups is harmless when each
# group has its own counter).  "sm"=misc(3 DMAs), "sq0-3"=wqk per head pair,
# "sx0-3"=x per 512-token block, "svv"=wv, "swp"=wp, "sst"=output stores.
LOAD_SEMS = ["sm", "sq0", "sq1", "sq2", "sq3", "sx0", "sx1", "sx2", "sx3",
             "svv", "swp"]

_CACHED = {}


def _plan(n_iter=1):
    """Build per-engine programs (descriptor lists, order == program order)
    with symbolic waits, then resolve to semaphore thresholds.

    n_iter > 1 repeats the compute (not the loads) with a coarse all-engine
    barrier between iterations -- used only for timing, so the marginal
    per-iteration time can be measured with the axon dispatch overhead
    cancelled."""
    sp, pe, act, dve = [], [], [], []
    act_idx, dve_idx, pe_unit = {}, {}, {}
    n_units = [0]
    n_exp = [0]
    n_class = [0]
    n_store = [0]

    def pop(key, **kw):
        pe_unit[key] = n_units[0]
        n_units[0] += 1
        pe.append({"key": key, "waits": kw.pop("waits", []), **kw})

    def aop(key, **kw):
        act_idx[key] = len(act)
        act.append({"key": key, "waits": kw.pop("waits", []), **kw})

    def dop(key, **kw):
        dve_idx[key] = len(dve)
        dve.append({"key": key, "waits": kw.pop("waits", []), **kw})

    # load issue order tuned for earliest first matmul: misc is tiny, then
    # the token-0 x block + pair-0 weights unblock qk(p0); wv unblocks the
    # first v groups; later x blocks feed the interleaved v matmuls
    for name, sem in ([("masks", "sm"), ("bqk", "sm"), ("bones", "sm"),
                       ("x0", "sx0"), ("wqk0", "sq0"), ("wv", "svv"),
                       ("wqk1", "sq1"), ("wqk2", "sq2"), ("wqk3", "sq3"),
                       ("x1", "sx1"), ("x2", "sx2"), ("x3", "sx3"),
                       ("wp", "swp")]):
        sp.append({"key": ("load", name), "kind": "load", "name": name,
                   "sem": sem, "waits": []})

    def class_slot():
        # psum halves rotate over 4 slots: (pair, half) = (slot//2, slot%2);
        # consumers (ACT ops) retire one or two halves per op -- n_class
        # counts halves, ca thresholds count ACT ops, so track the mapping
        g = n_class[0]
        n_class[0] += 1
        return g % 4

    def walk_one(_barrier_unused):
        # tenant tracking: consumer (sem, key) of each resource's current
        # occupant (reset per iteration; the inter-iteration barrier covers
        # carryover)
        e_slot_consumer = [None] * N_ESLOT
        slot_consumer = [None, None, None, None]   # psc halves
        av_bank_consumer = {4: None, 5: None}
        ps7_consumer = [None]
        ps6_consumer = [None]
        osb_tenant_store = [None, None, None]

        # ones column of v (softmax denominators) via DVE memset
        dop(("vmemset",), kind="vmemset", waits=[])

        def emit_v_upfront(vt):
            # v(0..3) run through the psav double buffer before attention
            bank = 4 + (vt % 2)
            w = [("svv#", 16), ("sx0#", 16)]
            if av_bank_consumer[bank]:
                w.append(av_bank_consumer[bank])
            pop(("v", vt), kind="vgrp", vt=vt, bank=bank, waits=w)
            av_bank_consumer[bank] = ("dv", ("vcopy", vt))
            dop(("vcopy", vt), kind="vcopy", vt=vt, bank=bank,
                waits=[("pc", ("v", vt))])

        # v(4..15) accumulate in ps6, one matmul at a time, sprinkled
        # between attention slots (emit_vmms below) so the PE never idles
        pending_vmm = [(vt, d) for vt in range(4, NKT) for d in range(DK)]
        vmm_pos = [0]

        def emit_vmms(k, keep=0):
            for _ in range(k):
                if vmm_pos[0] >= len(pending_vmm) - keep:
                    return
                vt, d = pending_vmm[vmm_pos[0]]
                vmm_pos[0] += 1
                w = []
                if d == 0:
                    w = [("svv#", 16), (f"sx{vt // 4}#", 16)]
                    if ps6_consumer[0]:
                        w.append(ps6_consumer[0])
                pop(("vmm", vt, d), kind="vmm", vt=vt, d=d,
                    start=(d == 0), stop=(d == DK - 1), waits=w)
                if d == DK - 1:
                    ps6_consumer[0] = ("dv", ("vcopy", vt))
                    dop(("vcopy", vt), kind="vcopy6", vt=vt,
                        waits=[("pc", ("vmm", vt, DK - 1))])

        def emit_qk(jt, tt):
            slot = class_slot()
            w = [(f"sq{jt % NPAIR}#", 16), (f"sx{tt}#", 16)]
            if slot_consumer[slot]:
                w.append(slot_consumer[slot])
            pop(("qk", jt, tt), kind="qkgrp", jt=jt, tt=tt, slot=slot, waits=w)
            slot_consumer[slot] = ("ca", ("qkc", jt, tt))
            aop(("qkc", jt, tt), kind="qkcopy", jt=jt, tt=tt, slot=slot,
                waits=[("pc", ("qk", jt, tt)), ("sm#", 48)])

        n_proj = [0]

        def emit_proj(ti, nt):
            tq = ti // 4
            # projections emitted while ps6 still stages v groups run in ps7
            # only; once the interleaved v's are done (tq >= 3), alternate
            # ps7/ps6 so the store copy of one group overlaps the next matmul
            use6 = tq >= 2 and n_proj[0] % 2 == 1
            n_proj[0] += 1
            w = [("dv", ("ym", tq, 3, 1)), ("swp#", 16)]
            tenant = ps6_consumer if use6 else ps7_consumer
            if tenant[0]:
                w.append(tenant[0])
            pop(("pj", ti, nt), kind="pjgrp", ti=ti, nt=nt, use6=use6,
                waits=w)
            tenant[0] = ("dv", ("oc", ti, nt))
            slot = n_store[0] % 3
            wo = [("pc", ("pj", ti, nt))]
            if osb_tenant_store[slot] is not None:
                wo.append(("sst#", 16 * (n_store[0] - 2)))
            dop(("oc", ti, nt), kind="oc", ti=ti, nt=nt, slot=slot, use6=use6,
                waits=wo)
            osb_tenant_store[slot] = n_store[0]
            n_store[0] += 1
            sp.append({"key": ("os", ti, nt), "kind": "ostore", "ti": ti,
                       "nt": nt, "slot": slot, "sem": "sst",
                       "waits": [("dv", ("oc", ti, nt))]})

        # head: pair-0 projections first (unblocks the first sc/exp chain),
        # then the first v groups, then the remaining qk pairs
        emit_qk(0, 0)
        emit_qk(NPAIR, 0)
        for vt_ in range(4):
            emit_v_upfront(vt_)
        for p_ in range(1, NPAIR):
            emit_qk(p_, 0)
            emit_qk(NPAIR + p_, 0)

        # ---- attention, with qk(tt+1) and proj(tq-1) sprinkled between pairs
        for tq in range(NTQ):
            for p in range(NPAIR):
                it = tq * NPAIR + p
                n_kt = 4 * (tq + 1)
                bankA, bankB = 4, 5

                sc_slot, sc_tenant = {}, {}
                for kt in range(n_kt):
                    ps = class_slot() // 2 * 2  # A half; B = ps+1
                    class_slot()
                    sc_slot[kt] = ps
                    sc_tenant[kt] = (slot_consumer[ps], slot_consumer[ps + 1])
                    slot_consumer[ps] = ("ca", ("exp", tq, p, kt))
                    slot_consumer[ps + 1] = ("ca", ("exp", tq, p, kt))

                exp_slot = {}
                for kt in range(n_kt):
                    s = n_exp[0] % N_ESLOT
                    n_exp[0] += 1
                    exp_slot[kt] = s
                    w = [("pc", ("sc", tq, p, kt, 1)), ("sm#", 48)]
                    if e_slot_consumer[s]:
                        w.append(("pc", e_slot_consumer[s]))
                    aop(("exp", tq, p, kt), kind="exp", tq=tq, p=p,
                        kt=kt, slot=sc_slot[kt] // 2, eslot=s, waits=w)
                    e_slot_consumer[s] = ("av", tq, p, kt, 1)

                def emit_mask(kt):
                    # emitted lazily inside the kt loop so interleaved DVE
                    # work (vcopy6) never queues behind a mask whose exp is
                    # still waiting on later PE ops (deadlock otherwise)
                    dop(("mask", tq, p, kt), kind="mask", r=kt - 4 * tq,
                        eslot=exp_slot[kt],
                        waits=[("ca", ("exp", tq, p, kt)),
                               ("sm#", 48)])

                def emit_sc(kt):
                    for h in (0, 1):
                        half = sc_slot[kt] + h
                        w = [("ca", ("qkc", p, tq)),
                             ("ca", ("qkc", NPAIR + p, kt // 4))]
                        if sc_tenant[kt][h]:
                            w.append(sc_tenant[kt][h])
                        pop(("sc", tq, p, kt, h), kind="sc", tq=tq, p=p, kt=kt,
                            h=h, slot=half, waits=w)

                def emit_av(kt):
                    for h in (0, 1):
                        if kt >= 4 * tq:
                            w = [("dv", ("mask", tq, p, kt))]
                        else:
                            w = [("ca", ("exp", tq, p, kt))]
                        w.append(("dv", ("vcopy", kt)))
                        bank = bankA if h == 0 else bankB
                        if kt == 0 and av_bank_consumer[bank]:
                            w.append(av_bank_consumer[bank])
                        pop(("av", tq, p, kt, h), kind="av", tq=tq, p=p, kt=kt,
                            h=h, bank=bank, eslot=exp_slot[kt],
                            start=(kt == 0), stop=(kt == n_kt - 1), waits=w)

                # run scores two kt ahead of the AV chain (the psc double
                # pair-slot rotation holds exactly 2 kt) so PE has queued
                # matmul work while exp(kt) is still on ACT
                emit_sc(0)
                if n_kt > 1:
                    emit_sc(1)
                for kt in range(n_kt):
                    if kt + 2 < n_kt:
                        emit_sc(kt + 2)
                    if kt >= 4 * tq:
                        emit_mask(kt)
                    emit_av(kt)
                    # deferred v work rides in attention slots; rate chosen
                    # so each v block lands well before its first consumer
                    # (v[4t..4t+3] feed the diagonal of tq=t)
                    if tq == 0:
                        if p * 4 + kt >= 4:
                            emit_vmms(4)
                    elif tq in (1, 2):
                        # hold the last two v blocks back for tq3, whose
                        # ACT-bound slots otherwise leave the PE idle
                        emit_vmms(3 if tq == 1 else 2, keep=2 * DK)
                    else:
                        # drain everything in the first two slots: tq2's
                        # staggered projections start writing ps6 at kt==2,
                        # so no vmm may accumulate there after that
                        emit_vmms(DK)
                    # projections of the previous tq ride this tq's kt loop,
                    # spaced 4 slots apart so oc never gates the next pj
                    if tq >= 1 and kt == 2:
                        emit_proj((tq - 1) * 4 + p, 0)
                    if tq >= 1 and kt == 6:
                        emit_proj((tq - 1) * 4 + p, 1)
                av_bank_consumer[bankA] = ("dv", ("ym", tq, p, 0))
                av_bank_consumer[bankB] = ("dv", ("ym", tq, p, 1))

                for h in (0, 1):
                    dop(("recip", tq, p, h), kind="recip", h=h,
                        bank=(bankA if h == 0 else bankB),
                        waits=[("pc", ("av", tq, p, n_kt - 1, h))])

                if tq + 1 < NTQ:
                    emit_qk(p, tq + 1)
                    emit_qk(NPAIR + p, tq + 1)

                # bc replicates the recips into rows 64:128 of the same psav
                # bank (recip already consumed the denominator row, and ym
                # only reads rows 0:64) -- ps7 stays free for projections
                for h in (0, 1):
                    w = [("dv", ("recip", tq, p, h)), ("sm#", 48)]
                    pop(("bc", tq, p, h), kind="bc", h=h,
                        bank=(bankA if h == 0 else bankB), waits=w)
                for h in (0, 1):
                    dop(("bcc", tq, p, h), kind="bcc", h=h,
                        bank=(bankA if h == 0 else bankB),
                        waits=[("pc", ("bc", tq, p, h))])
                for h in (0, 1):
                    dop(("ym", tq, p, h), kind="ym", tq=tq, p=p, h=h,
                        bank=(bankA if h == 0 else bankB),
                        waits=[("dv", ("bcc", tq, p, h))])
            if tq == NTQ - 1:
                for gi in range(8):
                    emit_proj(tq * 4 + gi // 2, gi % 2)

    # ---- walk n_iter times, resolving each iteration's symbolic waits
    # immediately (so index maps hold the right generation)
    resolved_upto = {}

    def resolve(name, prog):
        for op in prog[resolved_upto.get(name, 0):]:
            out = []
            for sem, ref in op["waits"]:
                if sem.endswith("#"):
                    out.append((sem[:-1], ref))
                elif sem == "pc":
                    out.append(("pc", pe_unit[ref] + 1))
                elif sem == "ca":
                    out.append(("ca", act_idx[ref] + 1))
                elif sem == "dv":
                    out.append(("dv", dve_idx[ref] + 1))
                else:
                    raise AssertionError(sem)
            merged = {}
            for s, v in out:
                merged[s] = max(merged.get(s, 0), v)
            op["waits"] = [(s, v) for s, v in merged.items() if v > 0]
        resolved_upto[name] = len(prog)

    for i in range(n_iter):
        barrier = None
        if i > 0:
            barrier = {
                "pe": [("ca#", len(act)), ("dv#", len(dve))],
                "act": [("pc#", n_units[0]), ("dv#", len(dve))],
                "dve": [("pc#", n_units[0]), ("ca#", len(act)),
                        ("sst#", 16 * n_store[0])],
            }
        pre = {"pe": len(pe), "act": len(act), "dve": len(dve)}
        walk_one(barrier)
        if barrier:
            # attach the inter-iteration barrier to each engine's first new op
            for name, prog in (("pe", pe), ("act", act), ("dve", dve)):
                prog[pre[name]]["waits"] = (
                    list(barrier[name]) + prog[pre[name]]["waits"])
        for name, prog in (("sp", sp), ("pe", pe), ("act", act), ("dve", dve)):
            resolve(name, prog)

    for prog in (sp, pe, act, dve):
        last = {}
        for op in prog:
            kept = []
            for s, v in op["waits"]:
                if v > last.get(s, -1):
                    kept.append((s, v))
                    last[s] = v
            op["waits"] = kept

    return {"sp": sp, "pe": pe, "act": act, "dve": dve}


def _build_program(n_iter=1):
    import concourse.bass as bass
    import concourse.mybir as mybir
    from contextlib import ExitStack

    f32 = mybir.dt.float32
    bf16 = mybir.dt.bfloat16
    AF = mybir.ActivationFunctionType
    MUL = mybir.AluOpType.mult

    plan = _plan(n_iter)
    nc = bass.Bass()

    xT = nc.dram_tensor("xT", [D, S], bf16, kind="ExternalInput")
    wqk = nc.dram_tensor("wqk", [D, 2 * PD], bf16, kind="ExternalInput")
    wv = nc.dram_tensor("wv", [D, PD], bf16, kind="ExternalInput")
    wp = nc.dram_tensor("wp", [PD, D], bf16, kind="ExternalInput")
    bqk = nc.dram_tensor("bqk", [P, 9], f32, kind="ExternalInput")
    masks = nc.dram_tensor("masks", [P, 4, 512], bf16, kind="ExternalInput")
    bones = nc.dram_tensor("bones", [1, 64], bf16, kind="ExternalInput")
    out = nc.dram_tensor("out", [S, D], f32, kind="ExternalOutput")

    xT_r = xT.rearrange("(o p) t -> p o t", p=P)
    wqk_r = wqk.rearrange("(o p) j -> p o j", p=P)
    wv_r = wv.rearrange("(o p) j -> p o j", p=P)
    wp_r = wp.rearrange("(o p) n -> p o n", p=P)

    with ExitStack() as ctx:
        ctx.enter_context(
            nc.allow_low_precision(reason="softmax recips in bf16 (error ~0.2%)")
        )
        x_sb = ctx.enter_context(nc.sbuf_tensor([P, DK, S], bf16))
        wqk_sb = ctx.enter_context(nc.sbuf_tensor([P, DK, 2 * PD], bf16))
        wv_sb = ctx.enter_context(nc.sbuf_tensor([P, DK, PD], bf16))
        wp_sb = ctx.enter_context(nc.sbuf_tensor([P, PD // P, D], bf16))
        b_sb = ctx.enter_context(nc.sbuf_tensor([P, 9], f32))
        m_sb = ctx.enter_context(nc.sbuf_tensor([P, 4, 512], bf16))
        qkT_sb = ctx.enter_context(nc.sbuf_tensor([P, 2 * NPAIR, S], bf16))
        v_sb = ctx.enter_context(nc.sbuf_tensor([P, NKT, HPC, 65], bf16))
        y_sb = ctx.enter_context(nc.sbuf_tensor([P, NPAIR, S], bf16))
        e_sb = ctx.enter_context(nc.sbuf_tensor([P, N_ESLOT, 2, 512], bf16))
        ones_sb = ctx.enter_context(nc.sbuf_tensor([1, 64], bf16))
        r_sb = ctx.enter_context(nc.sbuf_tensor([1, 2, 512], bf16))
        bc_sb = ctx.enter_context(nc.sbuf_tensor([P, 512], f32))
        osb = ctx.enter_context(nc.sbuf_tensor([P, 3, 512], f32))
        psc01 = ctx.enter_context(nc.psum_tensor("psc01", [P, 2, 512], f32))
        psc23 = ctx.enter_context(nc.psum_tensor("psc23", [P, 2, 512], f32))

        def psc_half(s):
            return (psc01 if s < 2 else psc23)[:, s % 2, :]

        def psc_pair(pair):
            return (psc01 if pair == 0 else psc23)[:]

        psav = {b: ctx.enter_context(nc.psum_tensor(f"psav{b}", [P, 512], f32))
                for b in (4, 5)}
        ps6 = ctx.enter_context(nc.psum_tensor("ps6", [P, 512], f32))
        ps7 = ctx.enter_context(nc.psum_tensor("ps7", [P, 512], f32))

        sems = {}
        for s in ["pc", "ca", "dv", "sst"] + LOAD_SEMS:
            sems[s] = ctx.enter_context(nc.semaphore(s))
        pc, ca, dv = sems["pc"], sems["ca"], sems["dv"]
        block = ctx.enter_context(nc.Block())

        # wqk is staged host-side in pair-major column order: pair p holds
        # its q block at cols [256p, 256p+128) and its k block at
        # [256p+128, 256p+256), so each pair is one contiguous DMA
        load_map = {
            "masks": (m_sb[:], masks[:]),
            "bqk": (b_sb[:], bqk[:]),
            "bones": (ones_sb[:], bones[:]),
            "wv": (wv_sb[:], wv_r[:]),
            "wp": (wp_sb[:], wp_r[:]),
        }
        for tt in range(4):
            load_map[f"x{tt}"] = (x_sb[:, :, tt * 512:(tt + 1) * 512],
                                  xT_r[:, :, tt * 512:(tt + 1) * 512])
        for pr_ in range(NPAIR):
            load_map[f"wqk{pr_}"] = (
                wqk_sb[:, :, 256 * pr_:256 * (pr_ + 1)],
                wqk_r[:, :, 256 * pr_:256 * (pr_ + 1)],
            )

        def qk_off(jt):
            return 256 * (jt % NPAIR) + (0 if jt < NPAIR else 128)

        def do_waits(eng, op):
            for s, v in op["waits"]:
                eng.wait_ge(sems[s], v)

        @block.sync
        def _(eng):
            for op in plan["sp"]:
                do_waits(eng, op)
                if op["kind"] == "load":
                    dst, src = load_map[op["name"]]
                    eng.dma_start(dst, src).then_inc(sems[op["sem"]], 16)
                else:
                    ti, nt, sl = op["ti"], op["nt"], op["slot"]
                    eng.dma_start(
                        out[ti * P:(ti + 1) * P, nt * 512:(nt + 1) * 512],
                        osb[:, sl, :],
                    ).then_inc(sems[op["sem"]], 16)

        def mm_split(out_ap, lhsT, rhs, **kw):
            # standalone LDWEIGHTS + non-self-loading MATMUL: lets the PE's
            # reorder window pull the next weight load into the background
            # buffer while the current matmul streams (fused self-loading
            # Matmult serializes the ~107ns load with every matmul)
            nc.tensor.ldweights(lhsT, tile_position=kw.get("tile_position"))
            mm = nc.tensor.matmul(out_ap, lhsT, rhs, **kw)
            mm.ins.ldweights = False
            return mm

        @block.tensor
        def _(eng):
            for op in plan["pe"]:
                do_waits(eng, op)
                k = op["kind"]
                if k == "vgrp":
                    vt, bank = op["vt"], op["bank"]
                    for d in range(DK):
                        mm = mm_split(
                            psav[bank][:],
                            x_sb[:, d, vt * P:(vt + 1) * P],
                            wv_sb[:, d, :],
                            start=(d == 0), stop=(d == DK - 1),
                        )
                    mm.then_inc(pc, 1)
                elif k == "vmm":
                    vt, d = op["vt"], op["d"]
                    mm_split(
                        ps6[:],
                        x_sb[:, d, vt * P:(vt + 1) * P],
                        wv_sb[:, d, :],
                        start=op["start"], stop=op["stop"],
                    ).then_inc(pc, 1)
                elif k == "qkgrp":
                    jt, tt, sl = op["jt"], op["tt"], op["slot"]
                    off = qk_off(jt)
                    for d in range(DK):
                        mm = mm_split(
                            psc_half(sl),
                            wqk_sb[:, d, off:off + P],
                            x_sb[:, d, tt * 512:(tt + 1) * 512],
                            start=(d == 0), stop=(d == DK - 1),
                        )
                    mm.then_inc(pc, 1)
                elif k == "sc":
                    tq, p, kt, h, sl = op["tq"], op["p"], op["kt"], op["h"], op["slot"]
                    pr = slice(64 * h, 64 * h + 64)
                    mm_split(
                        psc_half(sl),
                        qkT_sb[pr, NPAIR + p, kt * P:(kt + 1) * P],
                        qkT_sb[pr, p, tq * 512:(tq + 1) * 512],
                        start=True, stop=True, tile_position=(64 * h, 0),
                    ).then_inc(pc, 1)
                elif k == "av":
                    kt, h, bank = op["kt"], op["h"], op["bank"]
                    p = op["p"]
                    mm_split(
                        psav[bank][0:65, :],
                        v_sb[:, kt, 2 * p + h, :],
                        e_sb[:, op["eslot"], h, :],
                        start=op["start"], stop=op["stop"],
                    ).then_inc(pc, 1)
                elif k == "bc":
                    h = op["h"]
                    nc.tensor.matmul(
                        psav[op["bank"]][64:128, :],
                        ones_sb[0:1, :],
                        r_sb[0:1, h, :],
                        start=True, stop=True,
                        tile_position=(0, 64),
                    ).then_inc(pc, 1)
                else:  # pjgrp
                    ti, nt = op["ti"], op["nt"]
                    dst = ps6 if op["use6"] else ps7
                    for m in range(PD // P):
                        mm = mm_split(
                            dst[:],
                            y_sb[:, m, ti * P:(ti + 1) * P],
                            wp_sb[:, m, nt * 512:(nt + 1) * 512],
                            start=(m == 0), stop=(m == PD // P - 1),
                        )
                    mm.then_inc(pc, 1)

        @block.scalar
        def _(eng):
            for op in plan["act"]:
                do_waits(eng, op)
                if op["kind"] == "qkcopy":
                    jt, tt, sl = op["jt"], op["tt"], op["slot"]
                    nc.scalar.activation(
                        qkT_sb[:, jt, tt * 512:(tt + 1) * 512], psc_half(sl),
                        AF.Identity, bias=b_sb[:, jt:jt + 1], scale=1.0,
                    ).then_inc(ca, 1)
                else:  # exp -- processes both heads of the pair slot at once
                    nc.scalar.activation(
                        e_sb[:, op["eslot"], :, :], psc_pair(op["slot"]),
                        AF.Exp, bias=b_sb[:, 8:9], scale=0.125,
                    ).then_inc(ca, 1)

        @block.vector
        def _(eng):
            for op in plan["dve"]:
                do_waits(eng, op)
                k = op["kind"]
                if k == "vmemset":
                    nc.vector.memset(v_sb[:, :, :, 64], 1.0).then_inc(dv, 1)

                elif k == "vcopy":
                    vt, bank = op["vt"], op["bank"]
                    nc.vector.tensor_copy(
                        v_sb[:, vt, :, 0:64],
                        psav[bank][:].rearrange("p (h d) -> p h d", h=HPC),
                    ).then_inc(dv, 1)
                elif k == "vcopy6":
                    vt = op["vt"]
                    nc.vector.tensor_copy(
                        v_sb[:, vt, :, 0:64],
                        ps6[:].rearrange("p (h d) -> p h d", h=HPC),
                    ).then_inc(dv, 1)
                elif k == "mask":
                    e = e_sb[:, op["eslot"], :, :]
                    mb = m_sb[:, op["r"], None, :].to_broadcast((P, 2, 512))
                    nc.vector.tensor_tensor(e, e, mb, MUL).then_inc(dv, 1)
                elif k == "recip":
                    nc.vector.reciprocal(
                        r_sb[0:1, op["h"], :], psav[op["bank"]][64:65, :]
                    ).then_inc(dv, 1)
                elif k == "bcc":
                    h = op["h"]
                    rows = slice(64 * h, 64 * h + 64)
                    nc.vector.tensor_copy(bc_sb[rows, :],
                                          psav[op["bank"]][64:128, :]
                                          ).then_inc(dv, 1)
                elif k == "ym":
                    tq, p, h, bank = op["tq"], op["p"], op["h"], op["bank"]
                    rows = slice(64 * h, 64 * h + 64)
                    nc.vector.tensor_tensor(
                        y_sb[rows, p, tq * 512:(tq + 1) * 512],
                        psav[bank][0:64, :], bc_sb[rows, :], MUL,
                    ).then_inc(dv, 1)
                else:  # oc
                    src = ps6 if op["use6"] else ps7
                    nc.vector.tensor_copy(osb[:, op["slot"], :], src[:]
                                          ).then_inc(dv, 1)

    return nc


def _get_nc(n_iter=1):
    key = f"nc{n_iter}"
    if key not in _CACHED:
        _CACHED[key] = _build_program(n_iter)
    return _CACHED[key]


def _masks_np():
    bf = ml_dtypes.bfloat16
    j = np.arange(P)[:, None, None]
    r = np.arange(4)[None, :, None]
    i = np.arange(512)[None, None, :]
    return np.ascontiguousarray(((r * P + j) <= i).astype(bf))


def kernel(x, w_attn, b_attn, w_proj, b_proj):
    from concourse import bass_utils

    bf = ml_dtypes.bfloat16
    nc = _get_nc()
    masks_np = _masks_np()
    bones_np = np.ones((1, 64), dtype=bf)

    x = np.asarray(x)
    w_attn = np.asarray(w_attn)
    b_attn = np.asarray(b_attn, dtype=np.float32)
    w_proj = np.asarray(w_proj)
    b_proj = np.asarray(b_proj, dtype=np.float32)

    in_maps = []
    corrections = []
    for c in range(8):
        b, g = c // 2, c % 2
        heads = np.arange(g * HPC, (g + 1) * HPC)
        cols = (heads[:, None] * HD + np.arange(HD)[None, :]).reshape(-1)  # [512]
        qk_cols = np.concatenate([cols, D + cols])
        # device staging order is pair-major: [q-pair0, k-pair0, q-pair1, ...]
        qk_cols_dev = np.concatenate([
            np.concatenate([cols[128 * pr:128 * (pr + 1)],
                            D + cols[128 * pr:128 * (pr + 1)]])
            for pr in range(NPAIR)
        ])
        bqk_np = np.zeros((P, 9), np.float32)
        bqk_np[:, 0:8] = b_attn[qk_cols].reshape(8, P).T
        bv = b_attn[2 * D + cols]
        corrections.append(bv @ w_proj[cols, :])
        in_maps.append({
            "xT": np.ascontiguousarray(x[b].T).astype(bf),
            "wqk": np.ascontiguousarray(w_attn[:, qk_cols_dev]).astype(bf),
            "wv": np.ascontiguousarray(w_attn[:, 2 * D + cols]).astype(bf),
            "wp": np.ascontiguousarray(w_proj[cols, :]).astype(bf),
            "bqk": bqk_np,
            "masks": masks_np,
            "bones": bones_np,
        })

    trace = bool(int(os.environ.get("KERNEL_TRACE", "0")))
    try:
        res = bass_utils.run_bass_kernel_spmd(
            nc, in_maps, core_ids=list(range(8)), trace=trace,
        )
    except Exception:
        # transient device wedges have been observed on this rig; retry once
        import time as _time
        _time.sleep(5)
        res = bass_utils.run_bass_kernel_spmd(
            nc, in_maps, core_ids=list(range(8)), trace=trace,
        )
    _CACHED["last_results"] = res
    _CACHED["last_in_maps"] = in_maps

    outs = [np.asarray(r["out"], dtype=np.float32) for r in res.results]
    full = np.stack([
        outs[2 * b] + outs[2 * b + 1]
        + corrections[2 * b][None, :] + corrections[2 * b + 1][None, :]
        for b in range(B)
    ])
    full += b_proj[None, None, :]
    return full.astype(np.float32)

